# revision 3
# baseline (speedup 1.0000x reference)
"""Trainium2 Bass kernel for nn_Chooser_32229434589352 (segment_reduce).

Reference computation (per token row of x [1048576, 64]):
    h = tanh(x@W1+b1); h = tanh(h@W2+b2); h = tanh(h@W3+b3)
    logits = (h@W4 + b4)[:, 0]
    e = exp(logits - max(logits)); out = e / segment_sum(e)   (2048 segments x 512)

Key algebraic simplifications (exact up to fp rounding):
  * softmax is shift invariant, b4 is a global constant and the global max is a
    global constant -> both cancel in e/segsum(e).  logits are bounded
    (|l| <= sum|W4|*1 + |b4| ~ 4.5) so exp never overflows.  Hence NO
    cross-core collective is needed at all; each core handles 256 segments.

Sharding: 8 cores x 131072 tokens (256 segments).  Host pre-transposes x into
feature-major layout so the device DMAs are fully contiguous.

Per-core device pipeline (F=512 positions per matmul, all fp32):
  ranges: 128 ranges of 1024 tokens (= 2 segments).  groups of 8 ranges.
  L1 [64->128]: row-tiled pair of K=64 matmuls (concurrent on PE subarrays)
  L2 [128->64]: col-tiled pair (two W2 copies in array cols)
  L3 [64->16]:  2-range block-diag + 4x col-tiled -> PSUM [128,512] per round
  L4 [16->1]:   8-range block-diag -> logits PSUM [8,512]
  tanh on ScalarE with fused per-partition bias (engine bottleneck ~1.6cy/tok)
  exp with accum_out gives per-segment sums; DVE reciprocal + broadcast mult.
"""

import os
import numpy as np

B = 2048
S = 512
D = 64
TOTAL = B * S
NCORES = 8
L = TOTAL // NCORES  # 131072 tokens per core
F = 512              # matmul free dim / segment length
NG = 16              # groups per core (8 ranges each)
NR = 128             # ranges per core
RLEN = 1024          # tokens per range

_NC_CACHE = {}


def _build_nc(n_groups=NG, debug=False):
    """Build the Bass module for one core (SPMD across 8)."""
    from contextlib import ExitStack
    from concourse import bacc, bass, tile, mybir

    f32 = mybir.dt.float32
    Act = mybir.ActivationFunctionType
    n_hr = n_groups * 4          # half-rounds: (g, t, h)
    nrows = n_groups * 8         # output rows (= ranges); 128 at full size
    out_cols = RLEN              # positions per range

    nc = bacc.Bacc(
        "TRN2", target_bir_lowering=False, debug=debug, num_devices=NCORES
    )

    x_dram = nc.dram_tensor("xdev", [n_hr, 128, 1024], f32, kind="ExternalInput").ap()
    w1 = nc.dram_tensor("w1d", [128, 128], f32, kind="ExternalInput").ap()
    w2 = nc.dram_tensor("w2d", [128, 128], f32, kind="ExternalInput").ap()
    w3 = nc.dram_tensor("w3d", [128, 128], f32, kind="ExternalInput").ap()
    w4 = nc.dram_tensor("w4d", [128, 128], f32, kind="ExternalInput").ap()
    b1 = nc.dram_tensor("b1d", [128, 1], f32, kind="ExternalInput").ap()
    b2 = nc.dram_tensor("b2d", [128, 1], f32, kind="ExternalInput").ap()
    b3 = nc.dram_tensor("b3d", [128, 1], f32, kind="ExternalInput").ap()
    out_dram = nc.dram_tensor("out", [nrows, out_cols], f32, kind="ExternalOutput").ap()

    with tile.TileContext(nc) as tc, ExitStack() as ctx:
        consts = ctx.enter_context(tc.tile_pool(name="consts", bufs=1))
        xt_pool = ctx.enter_context(tc.tile_pool(name="xt", bufs=3))
        h1_pool = ctx.enter_context(tc.tile_pool(name="h1", bufs=2))
        h2_pool = ctx.enter_context(tc.tile_pool(name="h2", bufs=2))
        h3_pool = ctx.enter_context(tc.tile_pool(name="h3", bufs=2))
        big_pool = ctx.enter_context(tc.tile_pool(name="big", bufs=1))
        ps1_pool = ctx.enter_context(tc.tile_pool(name="ps1", bufs=1, space="PSUM"))
        ps2_pool = ctx.enter_context(tc.tile_pool(name="ps2", bufs=1, space="PSUM"))
        ps3_pool = ctx.enter_context(tc.tile_pool(name="ps3", bufs=1, space="PSUM"))
        ps4_pool = ctx.enter_context(tc.tile_pool(name="ps4", bufs=1, space="PSUM"))

        w1t = consts.tile([128, 128], f32, tag="w1t")
        w2t = consts.tile([128, 128], f32, tag="w2t")
        w3t = consts.tile([128, 128], f32, tag="w3t")
        w4t = consts.tile([128, 128], f32, tag="w4t")
        b1t = consts.tile([128, 1], f32, tag="b1t")
        b2t = consts.tile([128, 1], f32, tag="b2t")
        b3t = consts.tile([128, 1], f32, tag="b3t")
        nc.sync.dma_start(out=w1t[:], in_=w1[:])
        nc.sync.dma_start(out=w2t[:], in_=w2[:])
        nc.sync.dma_start(out=w3t[:], in_=w3[:])
        nc.sync.dma_start(out=w4t[:], in_=w4[:])
        nc.sync.dma_start(out=b1t[:], in_=b1[:])
        nc.sync.dma_start(out=b2t[:], in_=b2[:])
        nc.sync.dma_start(out=b3t[:], in_=b3[:])

        logits = big_pool.tile([nrows, out_cols], f32, tag="logits")

        n_gb = n_groups // 4
        for gb in range(n_gb):
            for t in range(2):
                ps4 = ps4_pool.tile([32, F], f32)
                for g4 in range(4):
                    g = 4 * gb + g4
                    ps3 = ps3_pool.tile([128, F], f32)
                    h3s = h3_pool.tile([128, F], f32)
                    for h in range(2):
                        hr = ((gb * 2 + t) * 4 + g4) * 2 + h
                        xt = xt_pool.tile([128, 1024], f32)
                        nc.sync.dma_start(out=xt[:], in_=x_dram[hr])

                        # L1: ranges 8g+4h+{0,1,2,3}; row-tiled K=64 pairs
                        ps1 = ps1_pool.tile([128, 2048], f32)
                        for j in range(2):
                            for u in range(2):
                                k = 2 * j + u
                                nc.tensor.matmul(
                                    ps1[:, F * k : F * (k + 1)],
                                    w1t[64 * u : 64 * (u + 1), :],
                                    xt[64 * u : 64 * (u + 1), F * j : F * (j + 1)],
                                    start=True,
                                    stop=True,
                                    tile_position=(64 * u, 0),
                                )
                        h1 = h1_pool.tile([128, 2048], f32)
                        nc.scalar.activation(h1[:], ps1[:], Act.Tanh, bias=b1t[:])

                        # L2: col-tiled M=64 pairs -> [even range rows 0:64,
                        # odd range rows 64:128] per column block
                        ps2 = ps2_pool.tile([128, 1024], f32)
                        for p in range(2):
                            for u in range(2):
                                nc.tensor.matmul(
                                    ps2[64 * u : 64 * (u + 1), F * p : F * (p + 1)],
                                    w2t[:, 64 * u : 64 * (u + 1)],
                                    h1[:, F * (2 * p + u) : F * (2 * p + u + 1)],
                                    start=True,
                                    stop=True,
                                    tile_position=(0, 64 * u),
                                )
                        h2 = h2_pool.tile([128, 1024], f32)
                        nc.scalar.activation(h2[:], ps2[:], Act.Tanh, bias=b2t[:])

                        # L3: block-diag (2 ranges), 4x col-tiled per round
                        for p in range(2):
                            i = 2 * h + p
                            nc.tensor.matmul(
                                ps3[32 * i : 32 * (i + 1), :],
                                w3t[:, 32 * i : 32 * (i + 1)],
                                h2[:, F * p : F * (p + 1)],
                                start=True,
                                stop=True,
                                tile_position=(0, 32 * i),
                            )
                    # one tanh for the whole round: rows 16v = range 8g+v
                    nc.scalar.activation(h3s[:], ps3[:], Act.Tanh, bias=b3t[:])

                    # L4: shifted 8-range block-diag; 4 groups accumulate into
                    # one [32, F] tile so the SBUF copy is 32-aligned.
                    # (b4 dropped: softmax shift invariance)
                    nc.tensor.matmul(
                        ps4[:],
                        w4t[:, 32 * g4 : 32 * (g4 + 1)],
                        h3s[:],
                        start=(g4 == 0),
                        stop=(g4 == 3),
                    )
                nc.vector.tensor_copy(
                    logits[32 * gb : 32 * (gb + 1), F * t : F * (t + 1)], ps4[:]
                )

        # tail: e = exp(logits) with per-segment sums, out = e / denom
        e = big_pool.tile([nrows, out_cols], f32, tag="e")
        nseg = out_cols // F
        denom = consts.tile([nrows, nseg], f32, tag="denom")
        rden = consts.tile([nrows, nseg], f32, tag="rden")
        for seg in range(nseg):
            nc.scalar.activation(
                e[:, F * seg : F * (seg + 1)],
                logits[:, F * seg : F * (seg + 1)],
                Act.Exp,
                accum_out=denom[:, seg : seg + 1],
            )
        nc.vector.reciprocal(rden[:], denom[:])
        outt = big_pool.tile([nrows, out_cols], f32, tag="outt")
        for seg in range(nseg):
            nc.vector.tensor_tensor(
                outt[:, F * seg : F * (seg + 1)],
                e[:, F * seg : F * (seg + 1)],
                rden[:, seg : seg + 1].broadcast_to([nrows, F]),
                mybir.AluOpType.mult,
            )
        nc.sync.dma_start(out=out_dram[:], in_=outt[:])

    nc.compile()
    return nc


def _prep_core_x(xc, n_groups=NG):
    """[L, 64] fp32 -> [n_hr, 128, 1024] feature-major DMA tiles.

    hr = ((gb*2 + t)*4 + g4)*2 + h; range r = 8*(4*gb+g4)+4h+2j+u gets tokens
    [r*1024, (r+1)*1024); tile holds positions [512t, 512t+512) of ranges
    8g+4h+{0..3}: rows 64u+d, cols 512j+f.
    """
    ntok = n_groups * 8 * RLEN
    # token index = ((((gb*4+g4)*2+h)*2+j)*2+u)*1024 + t*512 + f
    A = xc[:ntok].reshape(n_groups // 4, 4, 2, 2, 2, 2, 512, 64)  # gb g4 h j u t f d
    xdev = A.transpose(0, 5, 1, 2, 4, 7, 3, 6)  # gb t g4 h u d j f
    return np.ascontiguousarray(
        xdev.reshape(n_groups * 4, 128, 1024), dtype=np.float32
    )


def _prep_weights(W1, b1, W2, b2, W3, b3, W4):
    w1d = np.concatenate([W1, W1], axis=0).astype(np.float32)  # [128,128]
    w2d = np.concatenate([W2, W2], axis=1).astype(np.float32)  # [128,128]
    w3d = np.zeros((128, 128), np.float32)
    for i in range(4):
        w3d[0:64, 32 * i : 32 * i + 16] = W3
        w3d[64:128, 32 * i + 16 : 32 * i + 32] = W3
    w4d = np.zeros((128, 128), np.float32)
    for g4 in range(4):
        for v in range(8):
            w4d[16 * v : 16 * (v + 1), 32 * g4 + 8 * g4 + v] = W4[:, 0]
    b1d = np.asarray(b1, np.float32).reshape(128, 1)
    b2d = np.concatenate([b2, b2]).astype(np.float32).reshape(128, 1)
    b3d = np.tile(np.asarray(b3, np.float32), 8).reshape(128, 1)
    return dict(w1d=w1d, w2d=w2d, w3d=w3d, w4d=w4d, b1d=b1d, b2d=b2d, b3d=b3d)


def _host_reference(x, sizes, W1, b1, W2, b2, W3, b3, W4, b4):
    h = np.tanh(x @ W1 + b1)
    h = np.tanh(h @ W2 + b2)
    h = np.tanh(h @ W3 + b3)
    logits = (h @ W4 + b4)[:, 0]
    e = np.exp(logits - logits.max())
    seg_ids = np.repeat(np.arange(sizes.shape[0]), sizes)
    dens = np.zeros(sizes.shape[0], e.dtype)
    np.add.at(dens, seg_ids, e)
    return (e / dens[seg_ids]).astype(np.float32)


def kernel(x, sizes, W1, b1, W2, b2, W3, b3, W4, b4):
    x = np.ascontiguousarray(np.asarray(x, dtype=np.float32))
    sizes = np.asarray(sizes, dtype=np.int32)
    W1, b1, W2, b2, W3, b3, W4, b4 = [
        np.asarray(a, dtype=np.float32) for a in (W1, b1, W2, b2, W3, b3, W4, b4)
    ]

    if not (
        sizes.shape == (B,)
        and np.all(sizes == S)
        and x.shape == (TOTAL, D)
    ):
        # Layout is specialized to uniform 512-token segments; fall back to a
        # host computation for any other configuration.
        out = _host_reference(x, sizes, W1, b1, W2, b2, W3, b3, W4, b4)
        return out, sizes

    from concourse.bass_utils import run_bass_kernel_spmd

    if "nc" not in _NC_CACHE:
        _NC_CACHE["nc"] = _build_nc()
    nc = _NC_CACHE["nc"]

    wmap = _prep_weights(W1, b1, W2, b2, W3, b3, W4)
    xs = x.reshape(NCORES, L, D)
    in_maps = [dict(wmap, xdev=_prep_core_x(xs[c])) for c in range(NCORES)]

    res = run_bass_kernel_spmd(nc, in_maps, list(range(NCORES)))
    out = np.concatenate(
        [np.asarray(res.results[c]["out"]).reshape(-1) for c in range(NCORES)]
    ).astype(np.float32)
    return out, sizes
